# revision 22
# baseline (speedup 1.0000x reference)
"""Trainium2 Bass kernel for nn_DenseLtownLocGcn (4-layer GNN message passing).

Host folds the 10-scale conv + first dense layer into a 19-tap conv (exact),
then the device runs, per layer: folded conv + MLP for nodes and edges in
channel-partition layout, AllGather of node features (+fused gate channel),
PE-transpose to entity-major, indirect-gather into a col-sorted padded slot
layout, segment-softmax sums via per-128-node selection matmuls, ReduceScatter
of partial segment sums, GRU, halo exchange for the next layer's sequence
convs; finally a graph-data-parallel MLP. One SPMD program on 8 cores; all
per-core variation flows through input tensors.
"""
import numpy as np

NCORES = 8
N, E, NG = 15500, 62000, 500
NGP = 504
NP = NGP * 31            # 15624 padded nodes
NS = NP // NCORES        # 1953 nodes/core
ES = E // NCORES         # 7750 edges/core
R = 9                    # conv radius (19 taps)
TZ = 510                 # conv z-tile width (windows are TZ+2 wide for pair trick; fp32r needs even free dims)
LAYER_DIMS = [(12, 100), (112, 100), (112, 200), (212, 200)]
CINH = [0, 100, 100, 200]
CPAD = [104, 104, 208, 208]   # cout+1 rounded to mult of 8 (32B collective alignment)
ST = (NP + 127) // 128   # 123 slot/node tiles
NSLOT = ST * 128
GPC = NGP // NCORES      # 63 graphs per core
NSP = NS + 1             # 1954: padded (even) node width for conv/GRU tiles
NODE_ZT = [TZ, TZ, TZ, NSP - 3 * TZ]   # [510,510,510,424]
EDGE_ZT = [TZ] * 15 + [ES - 15 * TZ]   # [510]*15 + [100]
EXT_N = 1976             # gT width: [0:9 halo][9:1962 own][1962:1971 halo][junk->1976]
EXT_E = 7776             # eT width: [0:9][9:7759][7759:7768][junk->7776]

_CACHE = {}


# ---------------------------------------------------------------- host math --

def _fold_conv(p, cin):
    A = np.zeros((19, cin, 64), np.float32)
    b64 = np.asarray(p['db1'], np.float32).copy()
    for i in range(10):
        cW = np.asarray(p[f'cW{i}'], np.float32)
        cb = np.asarray(p[f'cb{i}'], np.float32)
        dW1_i = np.asarray(p['dW1'], np.float32)[i * cW.shape[0]:(i + 1) * cW.shape[0]]
        b64 += cb @ dW1_i
        for j in range(2 * i + 1):
            A[(j - i) + R] += cW[:, :, j].T @ dW1_i
    return A, b64


def _im2col(xp, base, count):
    M = xp.shape[0]
    out = np.zeros((19, 12, count), np.float32)
    for s in range(-R, R + 1):
        lo = base + s
        a0, a1 = max(0, lo), min(M, lo + count)
        if a1 > a0:
            out[s + R, :, a0 - lo:a1 - lo] = xp[a0:a1].T
    return out.reshape(228, count)


def _prep_host(x, edge_attr, edge_index, params):
    H = {}
    row, col = edge_index[0].astype(np.int64), edge_index[1].astype(np.int64)

    deg = np.zeros(N, np.float32)
    np.add.at(deg, col, 1.0)
    dis = np.where(deg > 0, 1.0 / np.sqrt(np.maximum(deg, 1e-12)), 0.0).astype(np.float32)
    w_edge = (dis[row] * dis[col]).astype(np.float32)

    x_pad = np.zeros((NP, 12), np.float32)
    x_pad[:N] = x

    W = {}
    for li, (cin, cout) in enumerate(LAYER_DIMS):
        L = params['layers'][li]
        ch = CINH[li]
        for grp in ('node', 'edge'):
            A, b64 = _fold_conv(L[grp], cin)
            Bx = np.zeros((228, 64), np.float32)
            for s in range(19):
                Bx[s * 12:(s + 1) * 12] = A[s, ch:ch + 12]
            W[f'l{li}{grp[0]}_bx'] = Bx.reshape(2, 114, 64)
            if ch:
                nkc = ch // 100
                Wp = np.zeros((10, nkc, 100, 128), np.float32)
                for p in range(10):
                    Ah = A[2 * p, :ch]
                    Bh = A[2 * p + 1, :ch] if 2 * p + 1 < 19 else np.zeros((ch, 64), np.float32)
                    full = np.concatenate([Ah, Bh], axis=1)
                    for k in range(nkc):
                        Wp[p, k] = full[k * 100:(k + 1) * 100]
                W[f'l{li}{grp[0]}_wp'] = Wp
            W[f'l{li}{grp[0]}_b64'] = b64.reshape(64, 1)
            W[f'l{li}{grp[0]}_dw2'] = np.asarray(L[grp]['dW2'], np.float32)
            W[f'l{li}{grp[0]}_db2'] = np.asarray(L[grp]['db2'], np.float32).reshape(32, 1)
            ncc = cout // 100
            dW3 = np.asarray(L[grp]['dW3'], np.float32)
            W[f'l{li}{grp[0]}_dw3'] = dW3.reshape(32, ncc, 100).transpose(1, 0, 2).copy()
            W[f'l{li}{grp[0]}_db3'] = np.asarray(L[grp]['db3'], np.float32).reshape(ncc, 100, 1)
        W[f'l{li}_gw'] = np.tile(np.asarray(L['gW'], np.float32).reshape(cout // 100, 100, 1), (1, 1, 2))
        H[f'l{li}_gb'] = float(np.asarray(L['gb']).reshape(-1)[0])
        nj, nk = 3 * cout // 100, cout // 100
        Wih = np.asarray(L['Wih'], np.float32)
        Whh = np.asarray(L['Whh'], np.float32)
        wip = np.zeros((nj, nk, 100, 100), np.float32)
        whp = np.zeros((nj, nk, 100, 100), np.float32)
        for j in range(nj):
            for k in range(nk):
                wip[j, k] = Wih[j * 100:(j + 1) * 100, k * 100:(k + 1) * 100].T
                whp[j, k] = Whh[j * 100:(j + 1) * 100, k * 100:(k + 1) * 100].T
        W[f'l{li}_wih'] = wip
        W[f'l{li}_whh'] = whp
        bs = np.asarray(L['bih'], np.float32) + np.asarray(L['bhh'], np.float32)
        W[f'l{li}_bsum'] = bs.reshape(nj, 100, 1)
        W[f'l{li}_bih'] = np.asarray(L['bih'], np.float32).reshape(nj, 100, 1)
        W[f'l{li}_bhh'] = np.asarray(L['bhh'], np.float32).reshape(nj, 100, 1)
    mW1 = np.asarray(params['mW1'], np.float32)
    m0 = np.zeros((31, 100, 512), np.float32)
    m1 = np.zeros((31, 100, 512), np.float32)
    for i in range(31):
        m0[i] = mW1[i * 200:i * 200 + 100]
        m1[i] = mW1[i * 200 + 100:i * 200 + 200]
    W['mw1p0'], W['mw1p1'] = m0, m1
    W['mb1p'] = np.asarray(params['mb1'], np.float32).reshape(4, 128, 1)
    W['mw2p'] = np.asarray(params['mW2'], np.float32).reshape(4, 128, 32)
    W['mb2'] = np.asarray(params['mb2'], np.float32).reshape(32, 1)
    W['iota'] = np.tile(np.arange(128, dtype=np.float32)[None, :], (128, 1))
    W['ident'] = np.eye(128, dtype=np.float32)
    W['zp'] = np.zeros((128, 64), np.float32)
    H['W'] = W

    cores = []
    for c in range(NCORES):
        D = {}
        nb, eb = c * NS, c * ES
        D['x2'] = np.concatenate([_im2col(x_pad, nb, NS), np.zeros((228, NSP - NS), np.float32)], axis=1).reshape(2, 114, NSP)
        D['ea2'] = _im2col(edge_attr, eb, ES).reshape(2, 114, ES)
        le = np.arange(eb, eb + ES)
        order = np.argsort(col[le], kind='stable')
        se = le[order]
        scol = col[se]
        srow_t = np.zeros((128, ST), np.int32)
        seid_t = np.zeros((128, ST), np.int32)
        sw_t = np.zeros((128, ST), np.float32)
        sc_t = np.full((128, ST), -1.0, np.float32)
        tstart = np.searchsorted(scol, np.arange(ST) * 128)
        tend = np.searchsorted(scol, np.arange(ST) * 128 + 128)
        for t in range(ST):
            a, b = tstart[t], tend[t]
            cnt = b - a
            assert cnt <= 128, f"slot overflow tile {t}: {cnt}"
            srow_t[:cnt, t] = row[se[a:b]]
            seid_t[:cnt, t] = se[a:b] - eb
            sw_t[:cnt, t] = w_edge[se[a:b]]
            sc_t[:cnt, t] = (scol[a:b] - t * 128).astype(np.float32)
        D['srT'], D['seT'], D['swT'], D['scT'] = srow_t, seid_t, sw_t, sc_t
        for li in range(3):
            co = LAYER_DIMS[li][1]
            big = np.full(co, 10 * co * NCORES, np.int32)
            D[f'hl{li}'] = (((c - 1) * co + np.arange(co)).astype(np.int32) if c > 0 else big).reshape(co // 100, 100, 1)
            D[f'hr{li}'] = (((c + 1) * co + np.arange(co)).astype(np.int32) if c < NCORES - 1 else big).reshape(co // 100, 100, 1)
        cores.append(D)
    H['cores'] = cores
    return H


# ------------------------------------------------------------- bass program --

def _build_program(dbg=()):
    import concourse.bass as bass
    import concourse.mybir as mybir
    import concourse.tile as tile
    from concourse import bacc

    f32 = mybir.dt.float32
    f32r = mybir.dt.float32r
    i32 = mybir.dt.int32
    AF = mybir.ActivationFunctionType
    OP = mybir.AluOpType
    RG = [list(range(NCORES))]
    F32 = mybir.dt.float32

    nc = bacc.Bacc(None, num_devices=NCORES)

    def param(name, shape, dt=f32r):
        return nc.declare_dram_parameter(name, list(shape), dt, isOutput=False)

    P = {}
    P['x2'] = param('x2', (2, 114, NSP))
    P['ea2'] = param('ea2', (2, 114, ES))
    P['srT'] = param('srT', (128, ST), i32)
    P['seT'] = param('seT', (128, ST), i32)
    P['swT'] = param('swT', (128, ST), f32)
    P['scT'] = param('scT', (128, ST), f32)
    P['iota'] = param('iota', (128, 128), f32)
    P['ident'] = param('ident', (128, 128))
    P['zp'] = param('zp', (128, 64))
    for li, (cin, cout) in enumerate(LAYER_DIMS):
        ch, ncc = CINH[li], cout // 100
        nj = 3 * cout // 100
        for g in 'ne':
            P[f'l{li}{g}_bx'] = param(f'l{li}{g}_bx', (2, 114, 64))
            if ch:
                P[f'l{li}{g}_wp'] = param(f'l{li}{g}_wp', (10, ch // 100, 100, 128))
            P[f'l{li}{g}_b64'] = param(f'l{li}{g}_b64', (64, 1), f32)
            P[f'l{li}{g}_dw2'] = param(f'l{li}{g}_dw2', (64, 32))
            P[f'l{li}{g}_db2'] = param(f'l{li}{g}_db2', (32, 1), f32)
            P[f'l{li}{g}_dw3'] = param(f'l{li}{g}_dw3', (ncc, 32, 100))
            P[f'l{li}{g}_db3'] = param(f'l{li}{g}_db3', (ncc, 100, 1), f32)
        P[f'l{li}_gw'] = param(f'l{li}_gw', (cout // 100, 100, 2))
        P[f'l{li}_wih'] = param(f'l{li}_wih', (nj, ncc, 100, 100))
        P[f'l{li}_whh'] = param(f'l{li}_whh', (nj, ncc, 100, 100))
        P[f'l{li}_bsum'] = param(f'l{li}_bsum', (nj, 100, 1), f32)
        P[f'l{li}_bih'] = param(f'l{li}_bih', (nj, 100, 1), f32)
        P[f'l{li}_bhh'] = param(f'l{li}_bhh', (nj, 100, 1), f32)
        if li < 3:
            P[f'hl{li}'] = param(f'hl{li}', (cout // 100, 100, 1), i32)
            P[f'hr{li}'] = param(f'hr{li}', (cout // 100, 100, 1), i32)
    P['mw1p0'] = param('mw1p0', (31, 100, 512))
    P['mw1p1'] = param('mw1p1', (31, 100, 512))
    P['mb1p'] = param('mb1p', (4, 128, 1), f32)
    P['mw2p'] = param('mw2p', (4, 128, 32))
    P['mb2'] = param('mb2', (32, 1), f32)

    out_ext = nc.declare_dram_parameter('out', [GPC, 32], f32, isOutput=True)
    dbg_ext = {}
    for (dl, what) in dbg:
        shp = {'gru': [LAYER_DIMS[dl][1], NS], 'h': [LAYER_DIMS[dl][1] + 1, NS],
               'seg': [NS, LAYER_DIMS[dl][1] + 1], 'e': [ES, LAYER_DIMS[dl][1] + 1]}[what]
        dbg_ext[(dl, what)] = nc.declare_dram_parameter(f'dbg_{what}{dl}', shp, f32, isOutput=True)

    DR = {}
    for li, (cin, cout) in enumerate(LAYER_DIMS):
        cp = CPAD[li]
        DR[f'h_my{li}'] = nc.dram_tensor(f'h_my{li}', [NS, cp], f32r)
        DR[f'h_full{li}'] = nc.dram_tensor(f'h_full{li}', [NP, cp], f32r, addr_space='Shared')
        DR[f'e_dram{li}'] = nc.dram_tensor(f'e_dram{li}', [ES, cp], f32r)
        DR[f'segp{li}'] = nc.dram_tensor(f'segp{li}', [NP, cp], f32r)
        DR[f'segs{li}'] = nc.dram_tensor(f'segs{li}', [NS, cp], f32r)
        if li < 3:
            DR[f'eT{li}'] = nc.dram_tensor(f'eT{li}', [cout, EXT_E], f32r)
            DR[f'gT{li}'] = nc.dram_tensor(f'gT{li}', [cout, EXT_N], f32r)
            DR[f'hnd_o{li}'] = nc.dram_tensor(f'hnd_o{li}', [cout, 18], f32r)
            DR[f'hnd_a{li}'] = nc.dram_tensor(f'hnd_a{li}', [NCORES * cout, 18], f32r, addr_space='Shared')
            DR[f'hed_o{li}'] = nc.dram_tensor(f'hed_o{li}', [cout, 18], f32r)
            DR[f'hed_a{li}'] = nc.dram_tensor(f'hed_a{li}', [NCORES * cout, 18], f32r, addr_space='Shared')

    gb_vals = _CACHE['gb_vals']

    with tile.TileContext(nc) as tc:
        with (
            tc.tile_pool(name='const', bufs=1) as cpool,
            tc.tile_pool(name='lw', bufs=1) as lwpool,
            tc.tile_pool(name='hT', bufs=1) as hpool,
            tc.tile_pool(name='work', bufs=2) as wpool,
            tc.tile_pool(name='gw', bufs=1) as gwpool,
            tc.tile_pool(name='slot', bufs=3) as spool,
            tc.tile_pool(name='ps', bufs=7, space='PSUM') as pp,
        ):
            def cload(tag, src, pool=cpool, bufs=None):
                t = pool.tile(list(src.shape), src.dtype, tag=f'c_{tag}', name=f'c_{tag}')
                nc.sync.dma_start(t[...], src)
                return t

            C = {}
            for nm in ('iota', 'ident', 'srT', 'seT', 'swT', 'scT'):
                C[nm] = cload(nm, P[nm][:, :])
            for nm in ('zp',):
                C[nm] = cload(nm, P[nm][:, :])
            for li, (cin, cout) in enumerate(LAYER_DIMS):
                ch, ncc = CINH[li], cout // 100
                nj = 3 * cout // 100
                for g in 'ne':
                    C[f'l{li}{g}_bx'] = [cload(f'l{li}{g}_bx{k}', P[f'l{li}{g}_bx'][k]) for k in range(2)]
                    for nm in ('b64', 'dw2', 'db2'):
                        C[f'l{li}{g}_{nm}'] = cload(f'l{li}{g}_{nm}', P[f'l{li}{g}_{nm}'][:, :])
                    C[f'l{li}{g}_dw3'] = [cload(f'l{li}{g}_dw3{k}', P[f'l{li}{g}_dw3'][k]) for k in range(ncc)]
                    C[f'l{li}{g}_db3'] = [cload(f'l{li}{g}_db3{k}', P[f'l{li}{g}_db3'][k]) for k in range(ncc)]
                C[f'l{li}_gw'] = [cload(f'l{li}_gw{cc}', P[f'l{li}_gw'][cc]) for cc in range(ncc)]
                for nm in ('bsum', 'bih', 'bhh'):
                    C[f'l{li}_{nm}'] = [cload(f'l{li}_{nm}{j}', P[f'l{li}_{nm}'][j]) for j in range(nj)]
                if li < 3:
                    C[f'hl{li}'] = [cload(f'hl{li}_{k}', P[f'hl{li}'][k]) for k in range(ncc)]
                    C[f'hr{li}'] = [cload(f'hr{li}_{k}', P[f'hr{li}'][k]) for k in range(ncc)]
            C['mw2p'] = [cload(f'mw2p{k}', P['mw2p'][k]) for k in range(4)]
            C['mb1p'] = [cload(f'mb1p{k}', P['mb1p'][k]) for k in range(4)]
            C['mb2'] = cload('mb2', P['mb2'][:, :])

            x4T = None

            def conv_tile(li, g, base, nz, hwin, xwin, dsts, ob, WP):
                """One conv z-tile + dense chain; writes [100,nz] channel chunks into
                dsts[0..ncc-1] at col offset ob; gate row into dsts[ncc-1] row 100."""
                cin, cout = LAYER_DIMS[li]
                ch, ncc = CINH[li], cout // 100
                zc = pp.tile([128, 512], F32, tag='ps', name='zc')
                first = True
                if ch:
                    for p in range(10):
                        for k in range(ch // 100):
                            nc.tensor.matmul(zc[:, 0:nz + 2], WP[p][k][:, :],
                                             hwin[k][:, 2 * p:2 * p + nz + 2],
                                             start=first, stop=False)
                            first = False
                for kx in range(2):
                    nc.tensor.matmul(zc[0:64, 0:nz], C[f'l{li}{g}_bx'][kx][:, :], xwin[kx],
                                     start=first, stop=(kx == 1))
                    first = False
                zt = wpool.tile([64, TZ], f32r, tag='zt', name='zt')
                if ch:
                    pbt = wpool.tile([64, TZ], F32, tag='pbt', name='pbt')
                    nc.scalar.activation(pbt[:, 0:nz], zc[64:128, 1:nz + 1], AF.Copy)
                    nc.vector.tensor_tensor(zt[:, 0:nz], zc[0:64, 0:nz], pbt[:, 0:nz], op=OP.add)
                    nc.scalar.activation(zt[:, 0:nz], zt[:, 0:nz], AF.Relu, bias=C[f'l{li}{g}_b64'][:, 0:1])
                else:
                    nc.scalar.activation(zt[:, 0:nz], zc[0:64, 0:nz], AF.Relu, bias=C[f'l{li}{g}_b64'][:, 0:1])
                p32 = pp.tile([32, TZ], F32, tag='ps', name='p32')
                nc.tensor.matmul(p32[:, 0:nz], C[f'l{li}{g}_dw2'][:, :], zt[:, 0:nz], start=True, stop=True)
                yt = wpool.tile([32, TZ], f32r, tag='yt', name='yt')
                nc.scalar.activation(yt[:, 0:nz], p32[:, 0:nz], AF.Relu, bias=C[f'l{li}{g}_db2'][:, 0:1])
                for cc in range(ncc):
                    ph = pp.tile([100, TZ], F32, tag='ps', name='ph')
                    nc.tensor.matmul(ph[:, 0:nz], C[f'l{li}{g}_dw3'][cc][:, :], yt[:, 0:nz],
                                     start=True, stop=True)
                    nc.scalar.activation(dsts[cc][0:100, ob:ob + nz], ph[:, 0:nz], AF.Identity,
                                         bias=C[f'l{li}{g}_db3'][cc][:, 0:1])
                pg = pp.tile([2, TZ], F32, tag='ps', name='pg')
                for cc in range(ncc):
                    nc.tensor.matmul(pg[:, 0:nz], C[f'l{li}_gw'][cc][:, :],
                                     dsts[cc][0:100, ob:ob + nz], start=(cc == 0), stop=(cc == ncc - 1))
                nc.scalar.activation(dsts[ncc][0:2, ob:ob + nz], pg[:, 0:nz], AF.Copy)

            def to_rows(srcs, sb, nn, cout, cp, dram_dst, drow):
                """PE-transpose col block [sb,sb+nn) of channel-major srcs (+ gate
                row tile last) into entity-major rows -> dram_dst[drow:drow+nn]."""
                ncc = cout // 100
                em = spool.tile([128, 208], f32r, tag='em', name='em')
                for cc in range(ncc):
                    pt = pp.tile([128, 100], f32r, tag='ps', name='pt')
                    nc.tensor.transpose(pt[0:nn, 0:100], srcs[cc][0:100, sb:sb + nn],
                                        C['ident'][0:100, 0:100])
                    nc.vector.tensor_copy(em[0:nn, cc * 100:cc * 100 + 100], pt[0:nn, 0:100])
                ptg = pp.tile([128, 100], f32r, tag='ps', name='ptg')
                nc.tensor.transpose(ptg[0:nn, 0:2], srcs[ncc][0:2, sb:sb + nn],
                                    C['ident'][0:2, 0:2])
                nc.vector.tensor_copy(em[0:nn, cout:cp],
                                      ptg[0:nn, 0:1].to_broadcast([nn, cp - cout]))
                nc.sync.dma_start(dram_dst[drow:drow + nn, 0:cp], em[0:nn, 0:cp])

            ML = _CACHE.get('max_layers', 4)
            PH = _CACHE.get('phases', 'agsr')
            for li, (cin, cout) in enumerate(LAYER_DIMS):
                if li >= ML:
                    break
                ch, ncc = CINH[li], cout // 100
                nj = 3 * cout // 100

                # per-layer weights (slot-shared tags across layers)
                WPn = WPe = None
                if ch:
                    WPn = [[cload(f'wpn{p}_{k}', P[f'l{li}n_wp'][p, k], pool=lwpool)
                            for k in range(ch // 100)] for p in range(10)]
                    WPe = [[cload(f'wpe{p}_{k}', P[f'l{li}e_wp'][p, k], pool=lwpool)
                            for k in range(ch // 100)] for p in range(10)]
                WIH = [[cload(f'wih{j}_{k}', P[f'l{li}_wih'][j, k], pool=lwpool) for k in range(ncc)]
                       for j in range(nj)]
                WHH = [[cload(f'whh{j}_{k}', P[f'l{li}_whh'][j, k], pool=lwpool) for k in range(ncc)]
                       for j in range(nj)]

                # ---------------- node conv ----------------
                hA = hpool.tile([100, NSP], f32r, tag='hA', name='hA')
                hB = hpool.tile([100, NSP], f32r, tag='hB', name='hB') if ncc > 1 else hA
                hG = hpool.tile([2, NSP], f32r, tag='hG', name='hG')
                hts = [hA, hB][:ncc] + [hG]
                base = 0
                for t, nz in enumerate(NODE_ZT):
                    hwin = None
                    if ch:
                        hwin = []
                        for k in range(ch // 100):
                            w = wpool.tile([100, TZ + 2 * R + 2], f32r, tag=f'ew{k}', name=f'ew{k}')
                            nc.sync.dma_start(w[:, 0:nz + 2 * R + 2],
                                              DR[f'gT{li - 1}'][k * 100:(k + 1) * 100,
                                                                base:base + nz + 2 * R + 2])
                            hwin.append(w)
                    xwin = []
                    for kx in range(2):
                        w = wpool.tile([114, TZ], f32r, tag=f'ex{kx}', name=f'ex{kx}')
                        nc.sync.dma_start(w[:, 0:nz], P['x2'][kx, :, base:base + nz])
                        xwin.append(w[:, 0:nz])
                    conv_tile(li, 'n', base, nz, hwin, xwin, hts, base, WPn)
                    base += nz

                for b0 in range(0, NS, 128):
                    nn = min(128, NS - b0)
                    to_rows(hts, b0, nn, cout, CPAD[li], DR[f'h_my{li}'], b0)
                if 'a' in PH:
                    nc.gpsimd.collective_compute(
                        'AllGather', OP.bypass, replica_groups=RG,
                        ins=[DR[f'h_my{li}'][:, :].bitcast(F32)],
                        outs=[DR[f'h_full{li}'][:, :].bitcast(F32)])
                if (li, 'h') in dbg_ext:
                    d = dbg_ext[(li, 'h')]
                    for cc in range(ncc):
                        nc.sync.dma_start(d[cc * 100:(cc + 1) * 100, :].bitcast(f32r), hts[cc][0:100, 0:NS])
                    nc.sync.dma_start(d[cout:cout + 1, :].bitcast(f32r), hts[ncc][0:1, 0:NS])

                # ---------------- edge conv ----------------
                base = 0
                for t, nz in enumerate(EDGE_ZT):
                    eA = spool.tile([100, TZ], f32r, tag='eA', name='eA')
                    eB = spool.tile([100, TZ], f32r, tag='eB', name='eB') if ncc > 1 else eA
                    eG = spool.tile([2, TZ], f32r, tag='eG', name='eG')
                    ets = [eA, eB][:ncc] + [eG]
                    hwin = None
                    if ch:
                        hwin = []
                        for k in range(ch // 100):
                            w = wpool.tile([100, TZ + 2 * R + 2], f32r, tag=f'ew{k}', name=f'ew{k}')
                            nc.sync.dma_start(w[:, 0:nz + 2 * R + 2],
                                              DR[f'eT{li - 1}'][k * 100:(k + 1) * 100,
                                                                base:base + nz + 2 * R + 2])
                            hwin.append(w)
                    xwin = []
                    for kx in range(2):
                        w = wpool.tile([114, TZ], f32r, tag=f'ex{kx}', name=f'ex{kx}')
                        nc.sync.dma_start(w[:, 0:nz], P['ea2'][kx, :, base:base + nz])
                        xwin.append(w[:, 0:nz])
                    conv_tile(li, 'e', base, nz, hwin, xwin, ets, 0, WPe)

                    if li < 3:
                        for cc in range(ncc):
                            nc.sync.dma_start(
                                DR[f'eT{li}'][cc * 100:(cc + 1) * 100, R + base:R + base + nz],
                                ets[cc][0:100, 0:nz])
                        if t == 0:
                            for cc in range(ncc):
                                nc.sync.dma_start(DR[f'hed_o{li}'][cc * 100:(cc + 1) * 100, 0:9],
                                                  ets[cc][0:100, 0:9])
                        if t == len(EDGE_ZT) - 1:
                            for cc in range(ncc):
                                nc.sync.dma_start(DR[f'hed_o{li}'][cc * 100:(cc + 1) * 100, 9:18],
                                                  ets[cc][0:100, nz - 9:nz])
                    for b0 in range(0, nz, 128):
                        nn = min(128, nz - b0)
                        to_rows(ets, b0, nn, cout, CPAD[li], DR[f'e_dram{li}'], base + b0)
                    base += nz
                if (li, 'e') in dbg_ext:
                    nc.sync.dma_start(dbg_ext[(li, 'e')][:, :].bitcast(f32r), DR[f'e_dram{li}'][:, 0:cout + 1])

                # ---------------- slot phase ----------------
                cp = CPAD[li]
                for t in range(ST if 's' in PH else 0):
                    nn = min(128, NP - t * 128)
                    hg = spool.tile([128, 208], f32r, tag='hg', name='hg')
                    nc.gpsimd.indirect_dma_start(
                        out=hg[:, 0:cp], out_offset=None, in_=DR[f'h_full{li}'][:, :],
                        in_offset=bass.IndirectOffsetOnAxis(ap=C['srT'][:, t:t + 1], axis=0))
                    nc.gpsimd.indirect_dma_start(
                        out=hg[:, 0:cp], out_offset=None, in_=DR[f'e_dram{li}'][:, :],
                        in_offset=bass.IndirectOffsetOnAxis(ap=C['seT'][:, t:t + 1], axis=0),
                        compute_op=OP.add)
                    gexp = spool.tile([128, 1], f32r, tag='gexp', name='gexp')
                    nc.scalar.activation(gexp[:, :], hg[:, cout:cout + 1], AF.Exp,
                                         scale=C['swT'][:, t:t + 1], bias=gb_vals[li])
                    sS = spool.tile([128, 1], F32, tag='sS', name='sS')
                    nc.vector.tensor_tensor(sS[:, :], gexp[:, :], C['swT'][:, t:t + 1], op=OP.mult)
                    Y = spool.tile([128, 208], f32r, tag='Y', name='Y')
                    nc.vector.tensor_scalar_mul(Y[:, 0:cout], hg[:, 0:cout], sS[:, 0:1])
                    nc.vector.tensor_copy(Y[:, cout:cp], gexp[:, 0:1].to_broadcast([128, cp - cout]))
                    S = spool.tile([128, 128], f32r, tag='S', name='S')
                    nc.vector.tensor_tensor(S[:, 0:nn], C['scT'][:, t:t + 1].to_broadcast([128, nn]),
                                            C['iota'][:, 0:nn], op=OP.is_equal)
                    psg = pp.tile([128, 208], F32, tag='ps', name='psg')
                    nc.tensor.matmul(psg[0:nn, 0:cp], S[:, 0:nn], Y[:, 0:cp], start=True, stop=True)
                    sgt = spool.tile([128, 208], f32r, tag='sgt', name='sgt')
                    nc.scalar.activation(sgt[0:nn, 0:cp], psg[0:nn, 0:cp], AF.Copy)
                    nc.sync.dma_start(DR[f'segp{li}'][t * 128:t * 128 + nn, :], sgt[0:nn, 0:cp])

                if 'r' not in PH:
                    continue
                nc.gpsimd.collective_compute(
                    'ReduceScatter', OP.add, replica_groups=RG,
                    ins=[DR[f'segp{li}'][:, :].bitcast(F32)],
                    outs=[DR[f'segs{li}'][:, :].bitcast(F32)])
                if (li, 'seg') in dbg_ext:
                    nc.sync.dma_start(dbg_ext[(li, 'seg')][:, :].bitcast(f32r), DR[f'segs{li}'][:, 0:cout + 1])

                # ---------------- aggr ----------------
                agT = [hpool.tile([100, NSP], f32r, tag=f'agT{k}', name=f'agT{k}') for k in range(ncc)]
                for b0 in range(0, NS, 128):
                    nn = min(128, NS - b0)
                    sg = spool.tile([128, 208], f32r, tag='sg_in', name='sg')
                    nc.sync.dma_start(sg[0:nn, 0:CPAD[li]], DR[f'segs{li}'][b0:b0 + nn, :])
                    dsafe = spool.tile([128, 1], F32, tag='dsafe', name='dsafe')
                    nc.vector.tensor_scalar_max(dsafe[0:nn, :], sg[0:nn, cout:cout + 1], 1e-30)
                    rec = spool.tile([128, 1], F32, tag='rec', name='rec')
                    nc.vector.reciprocal(rec[0:nn, :], dsafe[0:nn, :])
                    ag = spool.tile([128, 200], f32r, tag='ag', name='ag')
                    nc.vector.tensor_scalar_mul(ag[0:nn, 0:cout], sg[0:nn, 0:cout], rec[0:nn, 0:1])
                    nt = nn if nn % 2 == 0 else nn + 1
                    for cc in range(ncc):
                        pt = pp.tile([100, 128], f32r, tag='ps', name='pt2')
                        nc.tensor.transpose(pt[:, 0:nt], ag[0:nt, cc * 100:(cc + 1) * 100],
                                            C['ident'][0:nt, 0:nt])
                        nc.vector.tensor_copy(agT[cc][:, b0:b0 + nt], pt[:, 0:nt])

                # ---------------- GRU ----------------
                if li == 3:
                    x4T = [hpool.tile([100, NSP], f32r, tag=f'x4T{k}', name=f'x4T{k}')
                           for k in range(ncc)]
                base = 0
                for nz in NODE_ZT:
                    def gates(jj):
                        pgi = pp.tile([100, TZ], F32, tag='ps', name='pgi')
                        pgh = pp.tile([100, TZ], F32, tag='ps', name='pgh')
                        for k in range(ncc):
                            nc.tensor.matmul(pgi[:, 0:nz], WIH[jj][k][:, :],
                                             agT[k][:, base:base + nz], start=(k == 0), stop=(k == ncc - 1))
                            nc.tensor.matmul(pgh[:, 0:nz], WHH[jj][k][:, :],
                                             hts[k][0:100, base:base + nz], start=(k == 0), stop=(k == ncc - 1))
                        return pgi, pgh
                    rr, zz = [], []
                    for j in range(ncc):
                        pgi, pgh = gates(j)
                        r = gwpool.tile([100, TZ], F32, tag=f'g_r{j}', name=f'g_r{j}')
                        nc.scalar.activation(r[:, 0:nz], pgh[:, 0:nz], AF.Copy)
                        nc.vector.tensor_tensor(r[:, 0:nz], pgi[:, 0:nz], r[:, 0:nz], op=OP.add)
                        nc.scalar.activation(r[:, 0:nz], r[:, 0:nz], AF.Sigmoid,
                                             bias=C[f'l{li}_bsum'][j][:, 0:1])
                        rr.append(r)
                    for j in range(ncc):
                        pgi, pgh = gates(ncc + j)
                        z = gwpool.tile([100, TZ], F32, tag=f'g_z{j}', name=f'g_z{j}')
                        nc.scalar.activation(z[:, 0:nz], pgh[:, 0:nz], AF.Copy)
                        nc.vector.tensor_tensor(z[:, 0:nz], pgi[:, 0:nz], z[:, 0:nz], op=OP.add)
                        nc.scalar.activation(z[:, 0:nz], z[:, 0:nz], AF.Sigmoid,
                                             bias=C[f'l{li}_bsum'][ncc + j][:, 0:1])
                        zz.append(z)
                    for j in range(ncc):
                        jj = 2 * ncc + j
                        pgi, pgh = gates(jj)
                        tmp = gwpool.tile([100, TZ], F32, tag='g_tmp', name='g_tmp')
                        nc.scalar.activation(tmp[:, 0:nz], pgh[:, 0:nz], AF.Identity,
                                             bias=C[f'l{li}_bhh'][jj][:, 0:1])
                        nc.vector.tensor_tensor(tmp[:, 0:nz], rr[j][:, 0:nz], tmp[:, 0:nz], op=OP.mult)
                        nc.vector.tensor_tensor(tmp[:, 0:nz], pgi[:, 0:nz], tmp[:, 0:nz], op=OP.add)
                        cand = gwpool.tile([100, TZ], F32, tag='g_cand', name='g_cand')
                        nc.scalar.activation(cand[:, 0:nz], tmp[:, 0:nz], AF.Tanh,
                                             bias=C[f'l{li}_bih'][jj][:, 0:1])
                        nc.vector.tensor_tensor(tmp[:, 0:nz], hts[j][0:100, base:base + nz],
                                                cand[:, 0:nz], op=OP.subtract)
                        nc.vector.tensor_tensor(tmp[:, 0:nz], zz[j][:, 0:nz], tmp[:, 0:nz], op=OP.mult)
                        if li < 3:
                            go = gwpool.tile([100, TZ], f32r, tag='g_out', name='g_out')
                            nc.vector.tensor_tensor(go[:, 0:nz], cand[:, 0:nz], tmp[:, 0:nz], op=OP.add)
                            wnz = min(nz, NS - base)
                            nc.sync.dma_start(DR[f'gT{li}'][j * 100:(j + 1) * 100,
                                                            R + base:R + base + wnz], go[:, 0:wnz])
                        else:
                            nc.vector.tensor_tensor(x4T[j][:, base:base + nz], cand[:, 0:nz],
                                                    tmp[:, 0:nz], op=OP.add)
                    base += nz

                if (li, 'gru') in dbg_ext:
                    if li < 3:
                        nc.sync.dma_start(dbg_ext[(li, 'gru')][:, :].bitcast(f32r),
                                          DR[f'gT{li}'][0:cout, R:R + NS])
                    else:
                        for k in range(ncc):
                            nc.sync.dma_start(dbg_ext[(li, 'gru')][k * 100:(k + 1) * 100, :].bitcast(f32r),
                                              x4T[k][:, 0:NS])

                # ---------------- halo exchange ----------------
                if li < 3:
                    co = cout
                    for k in range(ncc):
                        nc.sync.dma_start(DR[f'hnd_o{li}'][k * 100:(k + 1) * 100, 0:9],
                                          DR[f'gT{li}'][k * 100:(k + 1) * 100, R:R + 9])
                        nc.sync.dma_start(DR[f'hnd_o{li}'][k * 100:(k + 1) * 100, 9:18],
                                          DR[f'gT{li}'][k * 100:(k + 1) * 100, R + NS - 9:R + NS])
                    nc.gpsimd.collective_compute('AllGather', OP.bypass, replica_groups=RG,
                                                 ins=[DR[f'hnd_o{li}'][:, :].bitcast(F32)],
                                                 outs=[DR[f'hnd_a{li}'][:, :].bitcast(F32)])
                    nc.gpsimd.collective_compute('AllGather', OP.bypass, replica_groups=RG,
                                                 ins=[DR[f'hed_o{li}'][:, :].bitcast(F32)],
                                                 outs=[DR[f'hed_a{li}'][:, :].bitcast(F32)])

                    for (idx_ts, colsl, left) in ((C[f'hl{li}'], slice(9, 18), True),
                                                  (C[f'hr{li}'], slice(0, 9), False)):
                        ndcol = 0 if left else R + NS
                        edcol = 0 if left else R + ES
                        for k in range(ncc):
                            tmp = spool.tile([100, 18], f32r, tag='halo_tmp', name='htmp')
                            nc.sync.dma_start(tmp[:, :], P['zp'][0:100, 0:18])
                            nc.gpsimd.indirect_dma_start(
                                out=tmp[:, :], out_offset=None, in_=DR[f'hnd_a{li}'][:, :],
                                in_offset=bass.IndirectOffsetOnAxis(ap=idx_ts[k][:, 0:1], axis=0),
                                bounds_check=NCORES * co - 1, oob_is_err=False)
                            nc.sync.dma_start(DR[f'gT{li}'][k * 100:(k + 1) * 100, ndcol:ndcol + 9],
                                              tmp[:, colsl])
                            tmp2 = spool.tile([100, 18], f32r, tag='halo_tmp', name='htmp2')
                            nc.sync.dma_start(tmp2[:, :], P['zp'][0:100, 0:18])
                            nc.gpsimd.indirect_dma_start(
                                out=tmp2[:, :], out_offset=None, in_=DR[f'hed_a{li}'][:, :],
                                in_offset=bass.IndirectOffsetOnAxis(ap=idx_ts[k][:, 0:1], axis=0),
                                bounds_check=NCORES * co - 1, oob_is_err=False)
                            nc.sync.dma_start(DR[f'eT{li}'][k * 100:(k + 1) * 100, edcol:edcol + 9],
                                              tmp2[:, colsl])
                    for k in range(ncc):
                        nc.sync.dma_start(DR[f'gT{li}'][k * 100:(k + 1) * 100, R + NS + 9:EXT_N],
                                          P['zp'][0:100, 0:EXT_N - R - NS - 9])
                        nc.sync.dma_start(DR[f'eT{li}'][k * 100:(k + 1) * 100, R + ES + 9:EXT_E],
                                          P['zp'][0:100, 0:EXT_E - R - ES - 9])

            # ---------------- final MLP ----------------
            if ML >= 4:
                pm = pp.tile([GPC, 512], F32, tag='ps', name='pm')
                first = True
                for i in range(31):
                    for cc in range(2):
                        mw = wpool.tile([100, 512], f32r, tag='mw1', name='mw')
                        nc.sync.dma_start(mw[:, :], P[f'mw1p{cc}'][i])
                        lhs = x4T[cc][:, 0:NS].rearrange('c (g i) -> c i g', i=31)[:, i, :]
                        nc.tensor.matmul(pm[:, :], lhs, mw[:, :], start=first,
                                         stop=(i == 30 and cc == 1))
                        first = False
                mcp = wpool.tile([64, 512], f32r, tag='mcp', name='mcp', bufs=1)
                nc.scalar.activation(mcp[0:GPC, :], pm[0:GPC, :], AF.Copy)
                po = pp.tile([32, 64], F32, tag='ps', name='po')
                for k in range(4):
                    pt = pp.tile([128, 64], f32r, tag='ps', name='ptm')
                    nc.tensor.transpose(pt[:, 0:64], mcp[:, k * 128:(k + 1) * 128],
                                        C['ident'][0:64, 0:64])
                    h1 = wpool.tile([128, 64], f32r, tag='h1', name='h1')
                    nc.scalar.activation(h1[:, :], pt[:, :], AF.Relu, bias=C['mb1p'][k][:, 0:1])
                    nc.tensor.matmul(po[:, :], C['mw2p'][k][:, :], h1[:, :],
                                     start=(k == 0), stop=(k == 3))
                outT = wpool.tile([32, 64], f32, tag='outT', name='outT')
                nc.scalar.activation(outT[:, :], po[:, :], AF.Identity, bias=C['mb2'][:, 0:1])
                nc.sync.dma_start(out_ext[:, :].rearrange('g o -> o g'), outT[:, 0:GPC])

    nc.compile()
    return nc


# ------------------------------------------------------------------- driver --

def kernel(**inputs):
    x = np.asarray(inputs['x'], np.float32)
    edge_attr = np.asarray(inputs['edge_attr'], np.float32)
    edge_index = np.asarray(inputs['edge_index'])
    params = inputs['params']

    H = _prep_host(x, edge_attr, edge_index, params)
    dbg = _CACHE.get('dbg', ())
    _CACHE['gb_vals'] = [H[f'l{li}_gb'] for li in range(4)]

    key = (dbg, _CACHE.get('max_layers', 4), _CACHE.get('phases', 'agsr'))
    if 'nc' not in _CACHE or _CACHE.get('nc_key') != key:
        _CACHE['nc'] = _build_program(dbg)
        _CACHE['nc_key'] = key

    W = H['W']
    in_maps = []
    for c in range(NCORES):
        D = H['cores'][c]
        m = dict(W)
        m['x2'], m['ea2'] = D['x2'], D['ea2']
        m['srT'], m['seT'] = D['srT'], D['seT']
        m['swT'], m['scT'] = D['swT'], D['scT']
        for li in range(3):
            m[f'hl{li}'], m[f'hr{li}'] = D[f'hl{li}'], D[f'hr{li}']
        in_maps.append(m)

    from concourse.bass_utils import run_bass_kernel_spmd
    res = run_bass_kernel_spmd(_CACHE['nc'], in_maps, list(range(NCORES)),
                               trace=_CACHE.get('trace', False))
    _CACHE['last_res'] = res
    out = np.concatenate([res.results[c]['out'] for c in range(NCORES)], axis=0)
    return out[:NG].astype(np.float32)
